# revision 1
# baseline (speedup 1.0000x reference)
"""Trainium2 Bass kernel for nn_BatchedMonomialFactor.

Math (per batch row b):
  logits = (x @ W_perm).reshape(R, B, B) / TAU
  soft   = sinkhorn_5(logits)            (5x row/col normalize, exp space)
  idx    = argmax_i soft[r, i, j]  -> hard one-hot over i
  h_perm[r, i] = sum_j [i == idx[r, j]] * h[r, j]
  out[r, i] = sigmoid(x@W_alpha)[r,i] * tanh(x@W_diag)[r,i] * h_perm[r,i]

Sharding: model-parallel over R (64 r-blocks -> 8 per core); every core
reads the full x_t, weights/h/out are sliced by r; no communication.
The forward output uses only the HARD permutation (straight-through),
and a positive per-column scale cannot change a column argmax, so the
final col-normalize of sinkhorn is skipped.

Engine split (pairs of 128-row batch tiles are fused into single ops
to halve Vector-engine instruction overhead): PE does the three matmuls
(fp32 for exact argmax fidelity); ACT does exp-eviction straight out of
PSUM (fused exp(2z)) plus the exps of the sigmoid/tanh path, which is
rewritten in exps so only one ACT table set is ever loaded; DVE does
the sinkhorn reduces/scales (its ~800us of 1x fp32 passes is the
critical path; no fused multiply+segmented-reduce op exists). GpSimd
offload of the normalize mults was measured 2026-08-08: plain and
broadcast TT mults DO run (is_equal crashes walrus codegen), but at
~4-5x the cost-model prediction (per-op ucode launch/drain overhead),
taking a rep to ~2.9ms -- route nothing to gpsimd (gps_phases param
kept for reference).
"""

from contextlib import ExitStack

import numpy as np

import concourse.bass as bass
import concourse.tile as tile
from concourse import bacc, mybir
from concourse.bass_utils import run_bass_kernel_spmd

N_CORES = 8
BATCH = 2048
D = 1024
R = 64
B = 16
TAU = 0.5
ITERS = 5

RG = R // N_CORES           # r-blocks per core = 8
NCOL = RG * B * B           # perm-logit cols per core = 2048
DCOL = RG * B               # diag/alpha cols per core = 128
P = 128                     # partitions
NT = BATCH // P             # batch tiles = 16
KT = D // P                 # contraction tiles = 8
F32 = mybir.dt.float32
AF = mybir.ActivationFunctionType
OP = mybir.AluOpType

# The ACT table-set chooser maps Exp -> exp_and_others and Ln ->
# natural_log (first set containing each func), which thrashes a ~2.7us
# table load on every exp<->ln switch. Our kernel only uses Exp and Ln;
# make natural_log_exp_and_others (which has both) the only candidate.
# Set ids are positional, so the dict keeps its original order/size.
import concourse.bacc as _bacc_mod
from concourse import hw_specs as _hw_specs

_orig_get_act_tables = _hw_specs.get_activation_tables


def _patched_get_act_tables(module_arch):
    tabs = _orig_get_act_tables(module_arch)
    return {
        name: (funcs if name == "natural_log_exp_and_others"
               else funcs - {AF.Exp, AF.Ln})
        for name, funcs in tabs.items()
    }


_bacc_mod.get_activation_tables = _patched_get_act_tables


def _build(reps=1, ablate=(), kbufs=3, sbufs=3, recip_eng='approx', tpg=2, xbufs=3,
           gps_phases=frozenset()):
    ablate = set(ablate)
    nc = bacc.Bacc("TRN2", target_bir_lowering=False, debug=False,
                   num_devices=N_CORES)
    xT = nc.dram_tensor("xT", [D, BATCH], F32, kind="ExternalInput")
    wp = nc.dram_tensor("wp", [D, NCOL], F32, kind="ExternalInput")
    wda = nc.dram_tensor("wda", [D, 2 * DCOL], F32, kind="ExternalInput")
    hs = nc.dram_tensor("hs", [BATCH, DCOL], F32, kind="ExternalInput")
    out = nc.dram_tensor("out", [BATCH, DCOL], F32, kind="ExternalOutput")

    with tile.TileContext(nc) as tc, ExitStack() as ctx:
        singles = ctx.enter_context(tc.tile_pool(name="singles", bufs=1))
        kpool = ctx.enter_context(tc.tile_pool(name="kpool", bufs=kbufs))
        small = ctx.enter_context(tc.tile_pool(name="small", bufs=sbufs))
        pspool = ctx.enter_context(tc.tile_pool(name="ps", bufs=2, space="PSUM"))

        # Resident operands: W_perm slice, [W_diag | W_alpha] slice.
        # Load the first 512-column chunk of every k first so the first
        # tile's matmuls can start while the rest streams in.
        wps, wdas = [], []
        for k in range(KT):
            w = singles.tile([P, NCOL], F32, tag=f"wp{k}")
            wps.append(w)
            w2 = singles.tile([P, 2 * DCOL], F32, tag=f"wda{k}")
            wdas.append(w2)
        for k in range(KT):
            nc.sync.dma_start(out=wps[k][:, 0:512],
                              in_=wp.ap()[k * P:(k + 1) * P, 0:512])
        # bulk weight streaming rides a different DMA queue (ScalarE's)
        # so the first tile's x/h loads on SyncE's queue aren't stuck
        # behind it.
        for k in range(KT):
            nc.scalar.dma_start(out=wdas[k][:],
                                in_=wda.ap()[k * P:(k + 1) * P, :])
        for k in range(KT):
            nc.scalar.dma_start(out=wps[k][:, 512:NCOL],
                                in_=wp.ap()[k * P:(k + 1) * P, 512:NCOL])
        xpool = ctx.enter_context(tc.tile_pool(name="xpool", bufs=xbufs))

        def act_recip(dst, src):
            if recip_eng == 'approx':
                # hw reciprocal() is an iterative divide (~7 cyc/elem,
                # measured 1808ns per [P,256] op); the seeded 2-ULP
                # Newton-Raphson variant is ~2.8x faster and 2 ULP is
                # below fp32 reorder noise, so argmax-safe. Inputs here
                # are positive sums -- no 0/denorm/inf edge cases.
                n = src.free_size()
                scr = small.tile([P, n], F32, tag=f"rscr{n}", name="rscr")
                nc.vector.reciprocal_approx_accurate(out=dst, in_=src,
                                                     scratch=scr)
                return
            if recip_eng == 'dve':
                nc.vector.reciprocal(out=dst, in_=src)
                return
            # 1/x = exp(-ln x); ln+exp share one ACT table set.
            n = src.free_size()
            tmp = small.tile([P, n], F32, tag=f"lntmp{n}", name="lntmp")
            nc.scalar.activation(out=tmp, in_=src, func=AF.Ln)
            nc.scalar.activation(out=dst, in_=tmp, func=AF.Exp, scale=-1.0)

        RGe = RG * tpg          # merged r-groups across tpg batch subtiles
        DCe = DCOL * tpg
        for bt in range((NT // tpg) * reps):
            bt = bt % (NT // tpg)

            # per-subtile x^T slices, streamed
            xts = []
            for s_ in range(tpg):
                xsub = []
                for k in range(KT):
                    xt = xpool.tile([P, P], F32, tag=f"xt{k}_{s_}")
                    nc.sync.dma_start(
                        out=xt,
                        in_=xT.ap()[k * P:(k + 1) * P,
                                    (bt * tpg + s_) * P:(bt * tpg + s_ + 1) * P])
                    xsub.append(xt)
                xts.append(xsub)

            K_t = kpool.tile([P, RGe, B, B], F32, tag="K")
            Kflat = K_t[:].rearrange("p g i j -> p (g i j)")

            # logits matmul in halves of 1024 (2 PSUM banks each);
            # evict through ACT with fused exp(2*z)  [1/TAU = 2].
            for s_ in range(tpg):
                for half in range(2):
                    ps = pspool.tile([P, 1024], F32, tag="psK")
                    for nb in range(2):
                        ncol0 = half * 1024 + nb * 512
                        for k in range(KT):
                            nc.tensor.matmul(
                                out=ps[:, nb * 512:(nb + 1) * 512],
                                lhsT=xts[s_][k][:],
                                rhs=wps[k][:, ncol0:ncol0 + 512],
                                start=(k == 0),
                                stop=(k == KT - 1),
                            )
                    nc.scalar.activation(
                            out=Kflat[:, (s_ * 2 + half) * 1024:
                                      (s_ * 2 + half + 1) * 1024],
                            in_=ps[:],
                            func=AF.Exp,
                            scale=2.0,
                        )

            # diag/alpha matmul: [x @ Wd | x @ Wa] -> one PSUM bank.
            psd = pspool.tile([P, tpg * 2 * DCOL], F32, tag="psD")
            for s_ in range(tpg):
                for k in range(KT):
                    nc.tensor.matmul(
                        out=psd[:, s_ * 2 * DCOL:(s_ + 1) * 2 * DCOL],
                        lhsT=xts[s_][k][:],
                        rhs=wdas[k][:],
                        start=(k == 0),
                        stop=(k == KT - 1),
                    )
            # sigmoid(a)*tanh(d) = (e2d - 1) / ((1 + e2d) * (1 + ena))
            e2d = small.tile([P, DCe], F32, tag="e2d")
            ena = small.tile([P, DCe], F32, tag="ena")
            for s_ in range(tpg):
                nc.scalar.activation(
                    out=e2d[:, s_ * DCOL:(s_ + 1) * DCOL],
                    in_=psd[:, s_ * 2 * DCOL:s_ * 2 * DCOL + DCOL],
                    func=AF.Exp, scale=2.0)
                nc.scalar.activation(
                    out=ena[:, s_ * DCOL:(s_ + 1) * DCOL],
                    in_=psd[:, s_ * 2 * DCOL + DCOL:(s_ + 1) * 2 * DCOL],
                    func=AF.Exp, scale=-1.0)
            num = small.tile([P, DCe], F32, tag="num")
            nc.vector.tensor_scalar_sub(out=num, in0=e2d, scalar1=1.0)
            den = small.tile([P, DCe], F32, tag="den")
            nc.vector.scalar_tensor_tensor(out=den, in0=e2d, scalar=1.0,
                                           in1=ena, op0=OP.add, op1=OP.mult)
            dpa = small.tile([P, DCe], F32, tag="dpa")
            # denom = (1+e2d)*(1+ena) = (e2d+1) + (e2d+1)*ena
            nc.vector.scalar_tensor_tensor(out=dpa, in0=e2d, scalar=1.0,
                                           in1=den, op0=OP.add, op1=OP.add)
            rden = small.tile([P, DCe], F32, tag="rden")
            act_recip(rden, dpa)
            dv = small.tile([P, DCe], F32, tag="dv")
            nc.vector.tensor_mul(out=dv, in0=num, in1=rden)

            def sinkhorn_final(g0, ng):
                # sinkhorn + hard-permutation + output for r-groups
                # [g0, g0+ng) of this tile's merged K. Splitting the first
                # tile into halves lets DVE start before all evictions land.
                Xs = K_t[:, g0:g0 + ng]                 # [P, ng, i, j]
                Xti = Xs.transpose([0, 1, 3, 2])        # [P, ng, j, i]
                DCs = ng * B
                csl = slice(g0 * B, (g0 + ng) * B)

                def bcast_gi(t):   # (g,i)-indexed -> broadcast over j
                    return (t[:].rearrange("p (g i) -> p g i", g=ng)
                            .unsqueeze(3).to_broadcast([P, ng, B, B]))

                def bcast_gj(t):   # (g,j)-indexed -> broadcast over i
                    return (t[:].rearrange("p (g j) -> p g j", g=ng)
                            .unsqueeze(2).to_broadcast([P, ng, B, B]))

                for it in range(ITERS):
                    rs = small.tile([P, DCs], F32, tag="rs")
                    nc.vector.reduce_sum(out=rs, in_=Xs,
                                         axis=mybir.AxisListType.X)
                    rr = small.tile([P, DCs], F32, tag="rr")
                    act_recip(rr, rs)
                    # normalize mults route to GpSimd (~0.52x DVE rate) to
                    # take them off the DVE critical path; reduces/recips
                    # cannot (gpsimd lacks free-dim reduce / is_equal).
                    eng = nc.gpsimd if (2 * it) in gps_phases else nc.vector
                    eng.tensor_tensor(out=Xs, in0=Xs, in1=bcast_gi(rr),
                                      op=OP.mult)
                    if it < ITERS - 1:
                        cs = small.tile([P, DCs], F32, tag="cs")
                        nc.vector.reduce_sum(out=cs, in_=Xti,
                                             axis=mybir.AxisListType.X)
                        rc = small.tile([P, DCs], F32, tag="rc")
                        act_recip(rc, cs)
                        eng = (nc.gpsimd if (2 * it + 1) in gps_phases
                               else nc.vector)
                        eng.tensor_tensor(out=Xs, in0=Xs,
                                          in1=bcast_gj(rc), op=OP.mult)

                # column max over i -> hard assignment mask -> h gather.
                M = small.tile([P, DCs], F32, tag="M")
                nc.vector.reduce_max(out=M, in_=Xti, axis=mybir.AxisListType.X)
                nc.vector.tensor_tensor(out=Xs, in0=Xs, in1=bcast_gj(M),
                                        op=OP.is_equal)
                nc.vector.tensor_tensor(out=Xs, in0=Xs,
                                        in1=bcast_gj(h_t[:, csl]), op=OP.mult)
                hp = small.tile([P, DCs], F32, tag="hp")
                nc.vector.reduce_sum(out=hp, in_=Xs, axis=mybir.AxisListType.X)
                nc.vector.tensor_mul(out=o_t[:, csl], in0=hp, in1=dv[:, csl])

            h_t = small.tile([P, DCe], F32, tag="h")
            for s_ in range(tpg):
                b0 = (bt * tpg + s_) * P
                nc.sync.dma_start(out=h_t[:, s_ * DCOL:(s_ + 1) * DCOL],
                                  in_=hs.ap()[b0:b0 + P, :])
            o_t = small.tile([P, DCe], F32, tag="o")

            if bt == 0:
                q = RGe // (2 * tpg)   # one eviction's worth of r-groups
                for s_ in range(2 * tpg):
                    sinkhorn_final(s_ * q, q)
            else:
                sinkhorn_final(0, RGe)

            for s_ in range(tpg):
                b0 = (bt * tpg + s_) * P
                nc.sync.dma_start(out=out.ap()[b0:b0 + P, :],
                                  in_=o_t[:, s_ * DCOL:(s_ + 1) * DCOL])

    nc.compile()
    return nc


_build_v1 = _build


def _sel_consts():
    """Selection matrices for the PE-side sinkhorn (v2 kernel).

    State A = 16 tiles of [128 W-cols, 512 batch]; tile t = (g = t>>1,
    h = t&1); partition p of a tile = (i_h = p>>4, j = p&15), i = h*8+i_h.
    Row sums live at RS row g*16+i, col sums at RS row g*16+j. SEL_t are
    the reduce lhsT ([k, q] one-hot, zero-padded so all 16 matmuls
    accumulate the same full PSUM region); NSEL_t are the negated
    broadcast lhsT ([q, p]).
    """
    k = np.arange(128)
    selr = np.zeros((16, 128, 128), np.float32)
    nselr = np.zeros((16, 128, 128), np.float32)
    selc = np.zeros((16, 128, 128), np.float32)
    ncsel = np.zeros((16, 128, 128), np.float32)
    for t in range(16):
        g, h = t >> 1, t & 1
        selr[t, k, g * 16 + h * 8 + (k >> 4)] = 1.0
        nselr[t, g * 16 + h * 8 + (k >> 4), k] = -1.0
        selc[t, k, g * 16 + (k & 15)] = 1.0
        ncsel[t, g * 16 + (k & 15), k] = -1.0
    return (selr.reshape(2048, 128), nselr.reshape(2048, 128),
            selc.reshape(2048, 128), ncsel.reshape(2048, 128),
            np.eye(128, dtype=np.float32))


F = 512                     # batch per stile
NST = BATCH // F            # stiles = 4
NTL = NCOL // P             # A tiles per stile = 16
KSPLIT = 3                  # update groups (of 2 tiles) on PE+ACT path (rest DVE)


def _build2(reps=1, ksplit=KSPLIT):
    """v2: log-space sinkhorn, A in SBUF, normalizers via PE matmuls.

    A = logits/tau (1/tau folded into W_perm host-side) in transposed
    layout [W-cols on partitions, batch free], one contiguous SBUF tile
    per 512-batch stile. Each of the 9 phases: ACT computes E=exp(A) (4
    big instrs), PE accumulates row/col sums of E into one PSUM tile via
    zero-padded selection matmuls (one shared ACT ln for all 64
    r-blocks), PE broadcasts -ln(sum) per tile; the A-update then splits
    across engines: ksplit tiles add A via an identity matmul into the
    same PSUM region and ACT copies back, the rest are DVE adds. The
    final phase transposes A9 back to batch-major on PE and DVE does
    only max / is_equal / h-gather. All PSUM accumulation groups are
    consecutive same-region (split-region accumulates silently reset).
    """
    nc = bacc.Bacc("TRN2", target_bir_lowering=False, debug=False,
                   num_devices=N_CORES)
    xT = nc.dram_tensor("xT", [D, BATCH], F32, kind="ExternalInput")
    wp = nc.dram_tensor("wp", [D, NCOL], F32, kind="ExternalInput")
    wda = nc.dram_tensor("wda", [D, 2 * DCOL], F32, kind="ExternalInput")
    hs = nc.dram_tensor("hs", [BATCH, DCOL], F32, kind="ExternalInput")
    selr_d = nc.dram_tensor("selr", [2048, 128], F32, kind="ExternalInput")
    nselr_d = nc.dram_tensor("nselr", [2048, 128], F32, kind="ExternalInput")
    selc_d = nc.dram_tensor("selc", [2048, 128], F32, kind="ExternalInput")
    ncsel_d = nc.dram_tensor("ncsel", [2048, 128], F32, kind="ExternalInput")
    ident_d = nc.dram_tensor("ident", [128, 128], F32, kind="ExternalInput")
    out = nc.dram_tensor("out", [BATCH, DCOL], F32, kind="ExternalOutput")

    with tile.TileContext(nc) as tc, ExitStack() as ctx:
        singles = ctx.enter_context(tc.tile_pool(name="singles", bufs=1))
        xpool = ctx.enter_context(tc.tile_pool(name="xpool", bufs=1))
        apool = ctx.enter_context(tc.tile_pool(name="apool", bufs=1))
        epool = ctx.enter_context(tc.tile_pool(name="epool", bufs=1))
        upool = ctx.enter_context(tc.tile_pool(name="upool", bufs=2))
        small = ctx.enter_context(tc.tile_pool(name="small", bufs=2))
        pkpool = ctx.enter_context(tc.tile_pool(name="pk", bufs=1, space="PSUM"))
        rspool = ctx.enter_context(tc.tile_pool(name="rs", bufs=1, space="PSUM"))
        ubpool = ctx.enter_context(tc.tile_pool(name="ub", bufs=2, space="PSUM"))
        pdpool = ctx.enter_context(tc.tile_pool(name="pd", bufs=1, space="PSUM"))
        tpool = ctx.enter_context(tc.tile_pool(name="tp", bufs=1, space="PSUM"))

        wps, wdas = [], []
        for k in range(KT):
            wps.append(singles.tile([P, NCOL], F32, tag=f"wp{k}", name=f"wp{k}"))
            wdas.append(singles.tile([P, 2 * DCOL], F32, tag=f"wda{k}",
                                     name=f"wda{k}"))
        selr_s = singles.tile([P, 16, P], F32, tag="selr")
        selc_s = singles.tile([P, 16, P], F32, tag="selc")
        nselr_s = singles.tile([P, 16, P], F32, tag="nselr")
        ncsel_s = singles.tile([P, 16, P], F32, tag="ncsel")
        ident_s = singles.tile([P, P], F32, tag="ident")

        nc.sync.dma_start(out=ident_s, in_=ident_d.ap()[:, :])
        for t in range(16):
            nc.sync.dma_start(out=selr_s[:, t], in_=selr_d.ap()[t * P:(t + 1) * P, :])
            nc.sync.dma_start(out=selc_s[:, t], in_=selc_d.ap()[t * P:(t + 1) * P, :])
            nc.sync.dma_start(out=nselr_s[:, t], in_=nselr_d.ap()[t * P:(t + 1) * P, :])
            nc.sync.dma_start(out=ncsel_s[:, t], in_=ncsel_d.ap()[t * P:(t + 1) * P, :])
        for k in range(KT):
            nc.sync.dma_start(out=wps[k][:, 0:256],
                              in_=wp.ap()[k * P:(k + 1) * P, 0:256])
        for k in range(KT):
            nc.scalar.dma_start(out=wdas[k][:],
                                in_=wda.ap()[k * P:(k + 1) * P, :])
        for k in range(KT):
            nc.scalar.dma_start(out=wps[k][:, 256:NCOL],
                                in_=wp.ap()[k * P:(k + 1) * P, 256:NCOL])

        for it in range(NST * reps):
            st = it % NST
            b0 = st * F

            xts = []
            for k in range(KT):
                xt = xpool.tile([P, F], F32, tag=f"xt{k}", name=f"xt{k}")
                nc.sync.dma_start(out=xt,
                                  in_=xT.ap()[k * P:(k + 1) * P, b0:b0 + F])
                xts.append(xt)

            A = apool.tile([P, NTL, F], F32, tag="A")
            for t in range(NTL):
                pk = pkpool.tile([P, F], F32, tag="pk")
                for k in range(KT):
                    nc.tensor.matmul(out=pk[:], lhsT=wps[k][:, t * P:(t + 1) * P],
                                     rhs=xts[k][:], start=(k == 0),
                                     stop=(k == KT - 1))
                nc.vector.tensor_copy(out=A[:, t], in_=pk[:])

            for ph in range(2 * ITERS - 1):
                row = (ph % 2 == 0)
                sel = selr_s if row else selc_s
                nsel = nselr_s if row else ncsel_s
                eqs = []
                for q in range(4):
                    Eq = epool.tile([P, 4, F], F32, tag=f"E{q}", name=f"E{q}")
                    nc.scalar.activation(out=Eq, in_=A[:, 4 * q:4 * (q + 1)],
                                         func=AF.Exp)
                    eqs.append(Eq)
                RS = rspool.tile([P, F], F32, tag="RS")
                for t in range(NTL):
                    nc.tensor.matmul(out=RS[:], lhsT=sel[:, t],
                                     rhs=eqs[t // 4][:, t % 4],
                                     start=(t == 0), stop=(t == NTL - 1))
                lnu = upool.tile([P, F], F32, tag="lnu")
                nc.scalar.activation(out=lnu, in_=RS[:], func=AF.Ln)
                # updates in groups of 4 tiles: one 4-bank Ub per
                # group; first kg groups add A on PE and copy back on ACT,
                # the rest are single DVE adds. Each Ub bank sees its
                # establishing bcast write, then (PE path) one region-
                # matched accumulate -- consecutive per bank, so safe.
                for grp in range(8):
                    Ub = ubpool.tile([P, 2, F], F32, tag="ub")
                    pe_path = grp < ksplit
                    for q in range(2):
                        t = grp * 2 + q
                        nc.tensor.matmul(out=Ub[:, q], lhsT=nsel[:, t],
                                         rhs=lnu[:], start=True,
                                         stop=not pe_path)
                    if pe_path:
                        for q in range(2):
                            t = grp * 2 + q
                            nc.tensor.matmul(out=Ub[:, q], lhsT=ident_s[:],
                                             rhs=A[:, t], start=False,
                                             stop=True, skip_group_check=True)
                        nc.scalar.copy(out=A[:, grp * 2:(grp + 1) * 2],
                                       in_=Ub[:])
                    else:
                        nc.vector.tensor_tensor(
                            out=A[:, grp * 2:(grp + 1) * 2], in0=A[:, grp * 2:(grp + 1) * 2],
                            in1=Ub[:], op=OP.add)

            for bi in range(4):
                bb = b0 + bi * P
                h_t = small.tile([P, DCOL], F32, tag="h")
                nc.sync.dma_start(out=h_t, in_=hs.ap()[bb:bb + P, :])
                psd = pdpool.tile([P, 2 * DCOL], F32, tag="psd")
                for k in range(KT):
                    nc.tensor.matmul(out=psd[:],
                                     lhsT=xts[k][:, bi * P:(bi + 1) * P],
                                     rhs=wdas[k][:], start=(k == 0),
                                     stop=(k == KT - 1))
                e2d = small.tile([P, DCOL], F32, tag="e2d")
                ena = small.tile([P, DCOL], F32, tag="ena")
                nc.scalar.activation(out=e2d, in_=psd[:, 0:DCOL], func=AF.Exp,
                                     scale=2.0)
                nc.scalar.activation(out=ena, in_=psd[:, DCOL:2 * DCOL],
                                     func=AF.Exp, scale=-1.0)
                num = small.tile([P, DCOL], F32, tag="num")
                nc.vector.tensor_scalar_sub(out=num, in0=e2d, scalar1=1.0)
                den = small.tile([P, DCOL], F32, tag="den")
                nc.vector.scalar_tensor_tensor(out=den, in0=e2d, scalar=1.0,
                                               in1=ena, op0=OP.add, op1=OP.mult)
                dpa = small.tile([P, DCOL], F32, tag="dpa")
                nc.vector.scalar_tensor_tensor(out=dpa, in0=e2d, scalar=1.0,
                                               in1=den, op0=OP.add, op1=OP.add)
                rden = small.tile([P, DCOL], F32, tag="rden")
                nc.vector.reciprocal(out=rden, in_=dpa)
                dv = small.tile([P, DCOL], F32, tag="dv")
                nc.vector.tensor_mul(out=dv, in0=num, in1=rden)

                opre = small.tile([P, DCOL], F32, tag="opre")
                for c in range(4):
                    a9t = tpool.tile([P, 512], F32, tag="a9t")
                    for q in range(4):
                        nc.tensor.transpose(
                            a9t[:, q * P:(q + 1) * P],
                            A[:, 4 * c + q, bi * P:(bi + 1) * P], ident_s[:])
                    v4 = a9t[:].rearrange("p (g i j) -> p g i j", g=2, i=B)
                    vt = v4.transpose([0, 1, 3, 2])
                    M = small.tile([P, 2, B], F32, tag="M")
                    nc.vector.reduce_max(out=M, in_=vt,
                                         axis=mybir.AxisListType.X)
                    mask = small.tile([P, 2, B, B], F32, tag="mask")
                    nc.vector.tensor_tensor(
                        out=mask, in0=v4,
                        in1=M[:].unsqueeze(2).to_broadcast([P, 2, B, B]),
                        op=OP.is_equal)
                    hb = (h_t[:, c * 32:(c + 1) * 32]
                          .rearrange("p (g j) -> p g j", g=2)
                          .unsqueeze(2).to_broadcast([P, 2, B, B]))
                    nc.vector.tensor_tensor(out=mask, in0=mask, in1=hb,
                                            op=OP.mult)
                    nc.vector.reduce_sum(
                        out=opre[:, c * 32:(c + 1) * 32]
                        .rearrange("p (g i) -> p g i", g=2),
                        in_=mask, axis=mybir.AxisListType.X)
                o_t = small.tile([P, DCOL], F32, tag="o")
                nc.vector.tensor_mul(out=o_t, in0=opre, in1=dv)
                nc.sync.dma_start(out=out.ap()[bb:bb + P, :], in_=o_t)

    nc.compile()
    return nc


_build = _build_v1

_NC = None


def _get_nc():
    global _NC
    if _NC is None:
        _NC = _build()
    return _NC


def kernel(x_t, h, W_perm, W_diag, W_alpha):
    x_t = np.ascontiguousarray(np.asarray(x_t, dtype=np.float32))
    h = np.asarray(h, dtype=np.float32)
    W_perm = np.asarray(W_perm, dtype=np.float32)
    W_diag = np.asarray(W_diag, dtype=np.float32)
    W_alpha = np.asarray(W_alpha, dtype=np.float32)

    xT = np.ascontiguousarray(x_t.T)                          # [D, BATCH]
    wp4 = W_perm.reshape(D, R, B * B)
    wd3 = W_diag.reshape(D, R, B)
    wa3 = W_alpha.reshape(D, R, B)
    h3 = h.reshape(BATCH, R, B)

    selr, nselr, selc, ncsel, ident = _sel_consts()
    in_maps = []
    for c in range(N_CORES):
        rsl = slice(c * RG, (c + 1) * RG)
        in_maps.append({
            "xT": xT,
            # NOTE: active v1 build applies 1/TAU in its exp eviction, so wp
            # stays unscaled here (v2 instead needs wp * (1/TAU) host-side).
            "wp": np.ascontiguousarray(wp4[:, rsl].reshape(D, NCOL)),
            "wda": np.ascontiguousarray(
                np.concatenate([wd3[:, rsl].reshape(D, DCOL),
                                wa3[:, rsl].reshape(D, DCOL)], axis=1)),
            "hs": np.ascontiguousarray(h3[:, rsl].reshape(BATCH, DCOL)),
            "selr": selr, "nselr": nselr, "selc": selc, "ncsel": ncsel,
            "ident": ident,
        })

    global _last_in_maps
    _last_in_maps = in_maps
    nc0 = _get_nc()
    expected = set()
    for alloc in nc0.m.functions[0].allocations:
        if isinstance(alloc, mybir.MemoryLocationSet) and alloc.kind == "ExternalInput":
            expected.add(alloc.memorylocations[0].name)
    run_maps = [{k: v for k, v in m.items() if k in expected} for m in in_maps]
    res = run_bass_kernel_spmd(nc0, run_maps, core_ids=list(range(N_CORES)))
    parts = [res.results[c]["out"].reshape(BATCH, RG, B) for c in range(N_CORES)]
    return np.concatenate(parts, axis=1).reshape(BATCH, R * B).astype(np.float32)

